# revision 7
# baseline (speedup 1.0000x reference)
"""Mixture-of-Depths routing kernel for Trainium2 (8 NeuronCores, SPMD).

Problem (per batch row b of 4):
    logits = x[b] @ W_router.T            # [4096]
    idx    = top_k(logits, 2048)          # half the tokens
    out[b] = x[b]; out[b][idx] = x[b][idx] @ W_block.T

Strategy: routing (router logits, top-k, gather, scatter) is pure data
movement / O(S*D) work and runs on the host in exact fp32 — the top-k
set it produces is bit-identical to the reference's (the boundary gap
between the K-th and (K+1)-th logit is ~5e-4 for every row, orders of
magnitude above fp32 matmul noise ~1e-6). Passthrough tokens are copied
from the original fp32 x, so they are EXACT.

The device does the one irreducible O(K*D^2) piece: a dense Linear over
the 8192 selected tokens, sharded 1024 tokens per core (exactly
balanced since top-k is a fixed count per row). Each core runs
y^T = W x^T with features on psum partitions and tokens on the free
axis: 8x8 chunk matmuls of [128,128]x[128,512], fp16 inputs / fp32
accumulate (rel err ~3e-4 vs the 2e-2 gate), psum drained to fp16 and
DMA'd out. PE time = 8192*1024*1024 MACs / 8 cores = 65536 cycles
(~27us @2.4GHz); DMA = 6MB/core (~17us @358GB/s) fully hidden.
"""
import os

import numpy as np

B, S, D = 4, 4096, 1024
K_TOP = 2048
N_CORES = 8
TPC = (B * K_TOP) // N_CORES   # 1024 selected tokens per core
NK = D // 128                  # 8 contraction / feature chunks
NG = TPC // 512                # 2 token groups of 512 (max moving free dim)

_cache: dict = {}


def _build_nc():
    import concourse.bass as bass
    import concourse.mybir as mybir
    from concourse.tile import TileContext

    class _SplitWaitTC(TileContext):
        """The walrus build in this container rejects instructions carrying
        more than one sync-wait command. Tile's wait assignment routinely
        attaches several. After scheduling, move excess waits onto
        single-wait NoOps inserted before the instruction on the same
        engine (engine streams execute in order, so semantics are kept)."""

        def __exit__(self, exc_type, exc_value, traceback):
            r = super().__exit__(exc_type, exc_value, traceback)
            if exc_type is None:
                uid = 0
                for fn in self.nc.m.functions:
                    for bb in fn.blocks:
                        out = []
                        for inst in bb.instructions:
                            si = inst.sync_info
                            if si is not None and len(si.on_wait) > 1:
                                waits = list(si.on_wait)
                                si.on_wait = waits[-1:]
                                for w in waits[:-1]:
                                    uid += 1
                                    out.append(
                                        mybir.InstNoOp(
                                            name=f"I-waitsplit-{uid}",
                                            engine=inst.engine,
                                            ins=[],
                                            outs=[],
                                            sync_info=mybir.SyncInfo(
                                                on_wait=[w], on_update=[]
                                            ),
                                            text_hint="waitsplit",
                                            bass_nofuse=True,
                                        )
                                    )
                            out.append(inst)
                        bb.instructions = out
            return r

    f32 = mybir.dt.float32
    f16 = mybir.dt.float16

    nc = bass.Bass("TRN2", target_bir_lowering=False, debug=False,
                   num_devices=N_CORES)
    xs_d = nc.dram_tensor("xs", [D, TPC], f16, kind="ExternalInput")
    # wtf row fs*128+p, col k*128+e = W[fs*128+e, k*128+p]: tile fs holds
    # the stationary blocks for ALL contraction chunks of output chunk fs
    wtf_d = nc.dram_tensor("wtf", [D, D], f16, kind="ExternalInput")
    yo_d = nc.dram_tensor("yo", [D, TPC], f16, kind="ExternalOutput")

    NF1 = 4   # fs chunks processed in phase 1 (k-outer, 8 psum banks)

    with _SplitWaitTC(nc) as tc:
        with (
            tc.tile_pool(name="xpool", bufs=1) as xpool,
            tc.tile_pool(name="wpool", bufs=1) as wpool,
            tc.tile_pool(name="cpool", bufs=1) as cpool,
            tc.tile_pool(name="opool", bufs=8) as opool,
            tc.tile_pool(name="mm_pool", bufs=8, space="PSUM") as mm_pool,
        ):
            # input streams: xs chunks on sync, wtf chunks on gpsimd; the
            # first (k=0, fs=0) matmul needs only the first tile of each
            xs = [xpool.tile([128, TPC], f16, name=f"xs{k}") for k in range(NK)]
            wtf = [wpool.tile([128, D], f16, name=f"wtf{f}") for f in range(NK)]
            for k in range(NK):
                nc.sync.dma_start(out=xs[k][:], in_=xs_d[k * 128:(k + 1) * 128, :])
                nc.gpsimd.dma_start(out=wtf[k][:],
                                    in_=wtf_d[k * 128:(k + 1) * 128, :])
            # first scalar/vector op triggers a ~1.3us ACT table load; eat it
            # during the input stream, off the critical path
            warm = cpool.tile([1, 128], f32)
            nc.vector.memset(warm[:], 0.0)
            warm2 = cpool.tile([1, 128], f32)
            nc.scalar.copy(out=warm2[:], in_=warm[:])

            def drain_store(fs, ps):
                # psum -> fp16 sbuf on scalar/vector, then out on sync/gpsimd
                of = opool.tile([128, TPC], f16, name="of")
                for g in range(NG):
                    dst = of[:, g * 512:(g + 1) * 512]
                    if (fs + g) % 2 == 0:
                        nc.scalar.copy(out=dst, in_=ps[g][:])
                    else:
                        nc.vector.tensor_copy(out=dst, in_=ps[g][:])
                eng = nc.sync if fs % 2 == 0 else nc.gpsimd
                eng.dma_start(out=yo_d[fs * 128:(fs + 1) * 128, :],
                              in_=of[:])

            # phase 1: k-outer over fs 0..NF1-1 across all 8 psum banks; PE
            # consumes each xs chunk as it lands (8 matmuls/chunk ~ arrival
            # cadence), so the PE never waits for the full input stream
            ps1 = [[mm_pool.tile([128, 512], f32, name="ps")
                    for g in range(NG)] for f in range(NF1)]
            for k in range(NK):
                for fs in range(NF1):
                    for g in range(NG):
                        nc.tensor.matmul(
                            out=ps1[fs][g][:],
                            lhsT=wtf[fs][:, k * 128:(k + 1) * 128],
                            rhs=xs[k][:, g * 512:(g + 1) * 512],
                            start=(k == 0), stop=(k == NK - 1))
            for fs in range(NF1):
                drain_store(fs, ps1[fs])

            # phase 2: fs-outer for the rest (inputs all resident); psum
            # banks recycle as the phase-1 drains complete
            for fs in range(NF1, NK):
                ps = [mm_pool.tile([128, 512], f32, name="ps")
                      for g in range(NG)]
                for k in range(NK):
                    for g in range(NG):
                        nc.tensor.matmul(
                            out=ps[g][:],
                            lhsT=wtf[fs][:, k * 128:(k + 1) * 128],
                            rhs=xs[k][:, g * 512:(g + 1) * 512],
                            start=(k == 0), stop=(k == NK - 1))
                drain_store(fs, ps)
    return nc


def _get_nc():
    if "nc" not in _cache:
        _cache["nc"] = _build_nc()
    return _cache["nc"]


def _route(x, W_router):
    """Host-side routing: exact fp32 logits -> per-row top-k index set."""
    wr = np.asarray(W_router, dtype=np.float32).reshape(D)
    logits = (x.reshape(B * S, D) @ wr).reshape(B, S)
    rows = []
    for b in range(B):
        idx = np.argpartition(logits[b], S - K_TOP)[S - K_TOP:]
        idx.sort()
        rows.append(b * S + idx)
    return np.concatenate(rows)          # [B*K_TOP] flat selected rows


def run(x, W_block, W_router, trace=False):
    from concourse.bass_utils import run_bass_kernel_spmd

    nc = _get_nc()
    x = np.asarray(x, dtype=np.float32)
    sel_rows = _route(x, W_router)
    xf = x.reshape(B * S, D)
    sel16 = xf[sel_rows].astype(np.float16)          # [8192, D]
    # wtf[fs*128+p, k*128+e] = W^T[k*128+p, fs*128+e]  (see _build_nc)
    wtT = np.asarray(W_block, dtype=np.float32).T.astype(np.float16)
    wtf = np.ascontiguousarray(
        wtT.reshape(NK, 128, NK, 128).transpose(2, 1, 0, 3).reshape(D, D))
    in_maps = []
    for c in range(N_CORES):
        chunk = sel16[c * TPC:(c + 1) * TPC]         # [TPC, D]
        in_maps.append({
            "xs": np.ascontiguousarray(chunk.T),     # [D, TPC] feature-major
            "wtf": wtf,
        })
    res = run_bass_kernel_spmd(nc, in_maps, core_ids=list(range(N_CORES)),
                               trace=trace)
    out = x.copy()
    outf = out.reshape(B * S, D)
    for c in range(N_CORES):
        yo = res.results[c]["yo"]                    # [D, TPC] f16
        outf[sel_rows[c * TPC:(c + 1) * TPC]] = yo.T.astype(np.float32)
    return out, res


def kernel(x, W_block, W_router, top_k):
    assert int(top_k) == K_TOP, f"kernel compiled for top_k={K_TOP}, got {top_k}"
    trace = bool(os.environ.get("MOD_TRACE"))
    out, _ = run(x, W_block, W_router, trace=trace)
    return out


# revision 8
# speedup vs baseline: 1.0409x; 1.0409x over previous
"""Mixture-of-Depths routing kernel for Trainium2 (8 NeuronCores, SPMD).

Problem (per batch row b of 4):
    logits = x[b] @ W_router.T            # [4096]
    idx    = top_k(logits, 2048)          # half the tokens
    out[b] = x[b]; out[b][idx] = x[b][idx] @ W_block.T

Strategy: routing (router logits, top-k, gather, scatter) is pure data
movement / O(S*D) work and runs on the host in exact fp32 — the top-k
set it produces matches the reference's (the boundary gap between the
K-th and (K+1)-th logit is ~5e-4 for every row, orders of magnitude
above fp32 matmul noise ~1e-6). Passthrough tokens are copied from the
original fp32 x, so they are EXACT.

The device does the one irreducible O(K*D^2) piece: a dense Linear over
the 8192 selected tokens, sharded 1024 tokens per core (exactly
balanced since top-k is a fixed count per row). Each core runs
y^T = W x^T with features on psum partitions and tokens on the free
axis, fp16 inputs / fp32 accumulate (rel err ~4e-4 vs the 2e-2 gate).

Schedule (trace-tuned):
- phase 1 is k-outer over output chunks fs0-3 across all 8 psum banks,
  so the PE consumes each arriving xs chunk at ~the DMA cadence and
  never stalls on the input stream; phase 2 (fs4-7) runs fs-outer with
  everything resident. PE density measured ~100% (216ns/512-col matmul).
- first input tiles are split/ordered so the first matmul only needs
  128KB+128KB; phase-2 weight halves stream behind phase-1 halves.
- psum is drained to fp16 by scalar+vector alternately; output DMAs go
  out per half-chunk on the sync/gpsimd queues as soon as drained.
PE time = 8192*1024*1024 MACs / 8 cores = 65536 cycles (~27.6us
@2.4GHz); DMA = 6MB/core fully hidden behind the PE.
"""
import os

import numpy as np

B, S, D = 4, 4096, 1024
K_TOP = 2048
N_CORES = 8
TPC = (B * K_TOP) // N_CORES   # 1024 selected tokens per core
NK = D // 128                  # 8 contraction / feature chunks
NG = TPC // 512                # 2 token groups of 512 (max moving free dim)
NF1 = 4                        # fs chunks in phase 1 (2 psum banks each)

_cache: dict = {}


def _build_nc():
    import concourse.bass as bass
    import concourse.mybir as mybir
    from concourse.tile import TileContext

    class _SplitWaitTC(TileContext):
        """The walrus build in this container rejects instructions carrying
        more than one sync-wait command. Tile's wait assignment routinely
        attaches several. After scheduling, move excess waits onto
        single-wait NoOps inserted before the instruction on the same
        engine (engine streams execute in order, so semantics are kept)."""

        def __exit__(self, exc_type, exc_value, traceback):
            r = super().__exit__(exc_type, exc_value, traceback)
            if exc_type is None:
                uid = 0
                for fn in self.nc.m.functions:
                    for bb in fn.blocks:
                        out = []
                        for inst in bb.instructions:
                            si = inst.sync_info
                            if si is not None and len(si.on_wait) > 1:
                                waits = list(si.on_wait)
                                si.on_wait = waits[-1:]
                                for w in waits[:-1]:
                                    uid += 1
                                    out.append(
                                        mybir.InstNoOp(
                                            name=f"I-waitsplit-{uid}",
                                            engine=inst.engine,
                                            ins=[],
                                            outs=[],
                                            sync_info=mybir.SyncInfo(
                                                on_wait=[w], on_update=[]
                                            ),
                                            text_hint="waitsplit",
                                            bass_nofuse=True,
                                        )
                                    )
                            out.append(inst)
                        bb.instructions = out
            return r

    f32 = mybir.dt.float32
    f16 = mybir.dt.float16

    nc = bass.Bass("TRN2", target_bir_lowering=False, debug=False,
                   num_devices=N_CORES)
    xs_d = nc.dram_tensor("xs", [D, TPC], f16, kind="ExternalInput")
    w_d = nc.dram_tensor("w", [D, D], f16, kind="ExternalInput")
    yo_d = nc.dram_tensor("yo", [D, TPC], f16, kind="ExternalOutput")

    with _SplitWaitTC(nc) as tc:
        with (
            tc.tile_pool(name="xpool", bufs=1) as xpool,
            tc.tile_pool(name="wpool", bufs=1) as wpool,
            tc.tile_pool(name="cpool", bufs=1) as cpool,
            tc.tile_pool(name="opool", bufs=8) as opool,
            tc.tile_pool(name="mm_pool", bufs=8, space="PSUM") as mm_pool,
        ):
            xs = [xpool.tile([128, TPC], f16, name=f"xs{k}") for k in range(NK)]
            wt = [wpool.tile([128, D], f16, name=f"wt{k}") for k in range(NK)]
            # sync ring: xs stream, first chunk split so the first matmul
            # only waits on 128KB
            nc.sync.dma_start(out=xs[0][:, 0:512], in_=xs_d[0:128, 0:512])
            nc.sync.dma_start(out=xs[0][:, 512:1024], in_=xs_d[0:128, 512:1024])
            for k in range(1, NK):
                nc.sync.dma_start(out=xs[k][:], in_=xs_d[k * 128:(k + 1) * 128, :])
            # gpsimd ring: phase-1 weight halves (cols 0:512 = fs0-3) first,
            # phase-2 halves stream behind
            for k in range(NK):
                nc.gpsimd.dma_start(out=wt[k][:, 0:512],
                                    in_=w_d[k * 128:(k + 1) * 128, 0:512])
            for k in range(NK):
                nc.gpsimd.dma_start(out=wt[k][:, 512:1024],
                                    in_=w_d[k * 128:(k + 1) * 128, 512:1024])
            # first scalar op triggers a ~1.3us ACT table load; eat it during
            # the input stream, off the critical path
            warm = cpool.tile([1, 128], f32)
            nc.vector.memset(warm[:], 0.0)
            warm2 = cpool.tile([1, 128], f32)
            nc.scalar.copy(out=warm2[:], in_=warm[:])

            def drain_store(fs, ps):
                # psum -> fp16 sbuf on scalar/vector, halves out on the
                # sync/gpsimd queues as soon as each is drained
                of = opool.tile([128, TPC], f16, name="of")
                for g in range(NG):
                    dst = of[:, g * 512:(g + 1) * 512]
                    if (fs + g) % 2 == 0:
                        nc.scalar.copy(out=dst, in_=ps[g][:])
                    else:
                        nc.vector.tensor_copy(out=dst, in_=ps[g][:])
                for g in range(NG):
                    eng = nc.sync if (fs + g) % 2 == 0 else nc.gpsimd
                    eng.dma_start(
                        out=yo_d[fs * 128:(fs + 1) * 128,
                                 g * 512:(g + 1) * 512],
                        in_=of[:, g * 512:(g + 1) * 512])

            # phase 1: k-outer over fs0-3 across all 8 psum banks
            ps1 = [[mm_pool.tile([128, 512], f32, name="ps")
                    for _ in range(NG)] for _ in range(NF1)]
            for k in range(NK):
                for fs in range(NF1):
                    for g in range(NG):
                        nc.tensor.matmul(
                            out=ps1[fs][g][:],
                            lhsT=wt[k][:, fs * 128:(fs + 1) * 128],
                            rhs=xs[k][:, g * 512:(g + 1) * 512],
                            start=(k == 0), stop=(k == NK - 1))
            for fs in range(NF1):
                drain_store(fs, ps1[fs])

            # phase 2: fs-outer for fs4-7; psum banks recycle as the
            # phase-1 drains complete
            for fs in range(NF1, NK):
                ps = [mm_pool.tile([128, 512], f32, name="ps")
                      for _ in range(NG)]
                for k in range(NK):
                    for g in range(NG):
                        nc.tensor.matmul(
                            out=ps[g][:],
                            lhsT=wt[k][:, fs * 128:(fs + 1) * 128],
                            rhs=xs[k][:, g * 512:(g + 1) * 512],
                            start=(k == 0), stop=(k == NK - 1))
                drain_store(fs, ps)
    return nc


def _get_nc():
    if "nc" not in _cache:
        _cache["nc"] = _build_nc()
    return _cache["nc"]


def _route(x, W_router):
    """Host-side routing: exact fp32 logits -> per-row top-k index set."""
    wr = np.asarray(W_router, dtype=np.float32).reshape(D)
    logits = (x.reshape(B * S, D) @ wr).reshape(B, S)
    rows = []
    for b in range(B):
        idx = np.argpartition(logits[b], S - K_TOP)[S - K_TOP:]
        idx.sort()
        rows.append(b * S + idx)
    return np.concatenate(rows)          # [B*K_TOP] flat selected rows


def run(x, W_block, W_router, trace=False):
    from concourse.bass_utils import run_bass_kernel_spmd

    nc = _get_nc()
    x = np.asarray(x, dtype=np.float32)
    sel_rows = _route(x, W_router)
    xf = x.reshape(B * S, D)
    sel16 = xf[sel_rows].astype(np.float16)          # [8192, D]
    wt16 = np.ascontiguousarray(
        np.asarray(W_block, dtype=np.float32).T.astype(np.float16))
    in_maps = []
    for c in range(N_CORES):
        chunk = sel16[c * TPC:(c + 1) * TPC]         # [TPC, D]
        in_maps.append({
            "xs": np.ascontiguousarray(chunk.T),     # [D, TPC] feature-major
            "w": wt16,
        })
    res = run_bass_kernel_spmd(nc, in_maps, core_ids=list(range(N_CORES)),
                               trace=trace)
    out = x.copy()
    outf = out.reshape(B * S, D)
    for c in range(N_CORES):
        yo = res.results[c]["yo"]                    # [D, TPC] f16
        outf[sel_rows[c * TPC:(c + 1) * TPC]] = yo.T.astype(np.float32)
    return out, res


def kernel(x, W_block, W_router, top_k):
    assert int(top_k) == K_TOP, f"kernel compiled for top_k={K_TOP}, got {top_k}"
    trace = bool(os.environ.get("MOD_TRACE"))
    out, _ = run(x, W_block, W_router, trace=trace)
    return out


# revision 9
# speedup vs baseline: 1.2253x; 1.1771x over previous
"""Mixture-of-Depths routing kernel for Trainium2 (8 NeuronCores, SPMD).

Problem (per batch row b of 4):
    logits = x[b] @ W_router.T            # [4096]
    idx    = top_k(logits, 2048)          # half the tokens
    out[b] = x[b]; out[b][idx] = x[b][idx] @ W_block.T

Strategy: routing (router logits, top-k, gather, scatter) is pure data
movement / O(S*D) work and runs on the host in exact fp32 — the top-k
set it produces matches the reference's (the boundary gap between the
K-th and (K+1)-th logit is ~5e-4 for every row, orders of magnitude
above fp32 matmul noise ~1e-6). Passthrough tokens are copied from the
original fp32 x, so they are EXACT.

The device does the one irreducible O(K*D^2) piece: a dense Linear over
the 8192 selected tokens, sharded 1024 tokens per core (exactly
balanced since top-k is a fixed count per row). Each core runs
y^T = W x^T with features on psum partitions and tokens on the free
axis, fp16 inputs / fp32 accumulate (rel err ~4e-4 vs the 2e-2 gate).

Schedule (trace-tuned):
- phase 1 is k-outer over output chunks fs0-3 across all 8 psum banks,
  so the PE consumes each arriving xs chunk at ~the DMA cadence and
  never stalls on the input stream; phase 2 (fs4-7) runs fs-outer with
  everything resident. PE density measured ~100% (216ns/512-col matmul).
- first input tiles are split/ordered so the first matmul only needs
  128KB+128KB; phase-2 weight halves stream behind phase-1 halves.
- psum is drained to fp16 by scalar+vector alternately; output DMAs go
  out per half-chunk on the sync/gpsimd queues as soon as drained.
- the PE starts HAM-throttled at half clock (4/8 gate); ~20 dummy
  matmuls on a scratch tile during the input stream warm it up so the
  real matmuls run at full rate from the first one.
- the Tile epilogue's redundant second barrier round is stripped
  post-build (round 1 already waits on every DMA-completion semaphore).
PE time = 8192*1024*1024 MACs / 8 cores = 65536 cycles (~27.6us
@2.4GHz); DMA = 6MB/core fully hidden behind the PE.
"""
import os

import numpy as np

B, S, D = 4, 4096, 1024
K_TOP = 2048
N_CORES = 8
TPC = (B * K_TOP) // N_CORES   # 1024 selected tokens per core
NK = D // 128                  # 8 contraction / feature chunks
NG = TPC // 512                # 2 token groups of 512 (max moving free dim)
NF1 = 4                        # fs chunks in phase 1 (2 psum banks each)
N_WARM = 20                    # HAM warm-up dummy matmuls

_cache: dict = {}


def _build_nc():
    import concourse.bass as bass
    import concourse.mybir as mybir
    from concourse.tile import TileContext

    class _SplitWaitTC(TileContext):
        """The walrus build in this container rejects instructions carrying
        more than one sync-wait command. Tile's wait assignment routinely
        attaches several. After scheduling, move excess waits onto
        single-wait NoOps inserted before the instruction on the same
        engine (engine streams execute in order, so semantics are kept)."""

        def __exit__(self, exc_type, exc_value, traceback):
            r = super().__exit__(exc_type, exc_value, traceback)
            if exc_type is None:
                uid = 0
                for fn in self.nc.m.functions:
                    for bb in fn.blocks:
                        out = []
                        for inst in bb.instructions:
                            si = inst.sync_info
                            if si is not None and len(si.on_wait) > 1:
                                waits = list(si.on_wait)
                                si.on_wait = waits[-1:]
                                for w in waits[:-1]:
                                    uid += 1
                                    out.append(
                                        mybir.InstNoOp(
                                            name=f"I-waitsplit-{uid}",
                                            engine=inst.engine,
                                            ins=[],
                                            outs=[],
                                            sync_info=mybir.SyncInfo(
                                                on_wait=[w], on_update=[]
                                            ),
                                            text_hint="waitsplit",
                                            bass_nofuse=True,
                                        )
                                    )
                            out.append(inst)
                        bb.instructions = out
                # strip the epilogue's second barrier round: round 1 already
                # waits on every engine + DMA-queue completion semaphore
                for fn in self.nc.m.functions:
                    bb = fn.blocks[-1]
                    insts = bb.instructions
                    pool_rel = [
                        idx for idx, i in enumerate(insts)
                        if isinstance(i, mybir.InstEventSemaphore)
                        and i.engine == mybir.EngineType.Pool
                        and i.sync_info is not None
                        and any(u.ant_name.endswith("_release")
                                for u in i.sync_info.on_update)]
                    if len(pool_rel) == 2:
                        bb.instructions = (insts[:pool_rel[0] + 1]
                                           + insts[pool_rel[1] + 1:])
            return r

    f32 = mybir.dt.float32
    f16 = mybir.dt.float16

    nc = bass.Bass("TRN2", target_bir_lowering=False, debug=False,
                   num_devices=N_CORES)
    xs_d = nc.dram_tensor("xs", [D, TPC], f16, kind="ExternalInput")
    w_d = nc.dram_tensor("w", [D, D], f16, kind="ExternalInput")
    yo_d = nc.dram_tensor("yo", [D, TPC], f16, kind="ExternalOutput")

    with _SplitWaitTC(nc) as tc:
        with (
            tc.tile_pool(name="xpool", bufs=1) as xpool,
            tc.tile_pool(name="wpool", bufs=1) as wpool,
            tc.tile_pool(name="cpool", bufs=1) as cpool,
            tc.tile_pool(name="opool", bufs=8) as opool,
            tc.tile_pool(name="mm_pool", bufs=8, space="PSUM") as mm_pool,
        ):
            xs = [xpool.tile([128, TPC], f16, name=f"xs{k}") for k in range(NK)]
            wt = [wpool.tile([128, D], f16, name=f"wt{k}") for k in range(NK)]
            # sync ring: xs stream, first chunk split so the first matmul
            # only waits on 128KB
            nc.sync.dma_start(out=xs[0][:, 0:512], in_=xs_d[0:128, 0:512])
            nc.sync.dma_start(out=xs[0][:, 512:1024], in_=xs_d[0:128, 512:1024])
            for k in range(1, NK):
                nc.sync.dma_start(out=xs[k][:], in_=xs_d[k * 128:(k + 1) * 128, :])
            # gpsimd ring: phase-1 weight halves (cols 0:512 = fs0-3) first,
            # phase-2 halves stream behind
            for k in range(NK):
                nc.gpsimd.dma_start(out=wt[k][:, 0:512],
                                    in_=w_d[k * 128:(k + 1) * 128, 0:512])
            for k in range(NK):
                nc.gpsimd.dma_start(out=wt[k][:, 512:1024],
                                    in_=w_d[k * 128:(k + 1) * 128, 512:1024])
            # first scalar op triggers a ~1.3us ACT table load; eat it during
            # the input stream, off the critical path
            warm = cpool.tile([1, 128], f32)
            nc.vector.memset(warm[:], 0.0)
            warm2 = cpool.tile([1, 128], f32)
            nc.scalar.copy(out=warm2[:], in_=warm[:])
            dum = cpool.tile([128, 256], f16)
            nc.vector.memset(dum[:], 0.25)

            def drain_store(fs, ps):
                # psum -> fp16 sbuf on scalar/vector, halves out on the
                # sync/gpsimd queues as soon as each is drained
                of = opool.tile([128, TPC], f16, name="of")
                for g in range(NG):
                    dst = of[:, g * 512:(g + 1) * 512]
                    if (fs + g) % 2 == 0:
                        nc.scalar.copy(out=dst, in_=ps[g][:])
                    else:
                        nc.vector.tensor_copy(out=dst, in_=ps[g][:])
                for g in range(NG):
                    eng = nc.sync if (fs + g) % 2 == 0 else nc.gpsimd
                    eng.dma_start(
                        out=yo_d[fs * 128:(fs + 1) * 128,
                                 g * 512:(g + 1) * 512],
                        in_=of[:, g * 512:(g + 1) * 512])

            # phase 1: k-outer over fs0-3 across all 8 psum banks
            ps1 = [[mm_pool.tile([128, 512], f32, name="ps")
                    for _ in range(NG)] for _ in range(NF1)]
            # HAM warm-up: dummy matmuls while the inputs stream (results
            # discarded; the first real matmul resets the bank via start)
            for _ in range(N_WARM):
                nc.tensor.matmul(out=ps1[0][0][:, 0:256],
                                 lhsT=dum[:, 0:128], rhs=dum[:, :],
                                 start=True, stop=True)
            for k in range(NK):
                for fs in range(NF1):
                    for g in range(NG):
                        nc.tensor.matmul(
                            out=ps1[fs][g][:],
                            lhsT=wt[k][:, fs * 128:(fs + 1) * 128],
                            rhs=xs[k][:, g * 512:(g + 1) * 512],
                            start=(k == 0), stop=(k == NK - 1))
            for fs in range(NF1):
                drain_store(fs, ps1[fs])

            # phase 2: fs-outer for fs4-7; psum banks recycle as the
            # phase-1 drains complete
            for fs in range(NF1, NK):
                ps = [mm_pool.tile([128, 512], f32, name="ps")
                      for _ in range(NG)]
                for k in range(NK):
                    for g in range(NG):
                        nc.tensor.matmul(
                            out=ps[g][:],
                            lhsT=wt[k][:, fs * 128:(fs + 1) * 128],
                            rhs=xs[k][:, g * 512:(g + 1) * 512],
                            start=(k == 0), stop=(k == NK - 1))
                drain_store(fs, ps)
    return nc


def _get_nc():
    if "nc" not in _cache:
        _cache["nc"] = _build_nc()
    return _cache["nc"]


def _route(x, W_router):
    """Host-side routing: exact fp32 logits -> per-row top-k index set."""
    wr = np.asarray(W_router, dtype=np.float32).reshape(D)
    logits = (x.reshape(B * S, D) @ wr).reshape(B, S)
    rows = []
    for b in range(B):
        idx = np.argpartition(logits[b], S - K_TOP)[S - K_TOP:]
        idx.sort()
        rows.append(b * S + idx)
    return np.concatenate(rows)          # [B*K_TOP] flat selected rows


def run(x, W_block, W_router, trace=False):
    from concourse.bass_utils import run_bass_kernel_spmd

    nc = _get_nc()
    x = np.asarray(x, dtype=np.float32)
    sel_rows = _route(x, W_router)
    xf = x.reshape(B * S, D)
    sel16 = xf[sel_rows].astype(np.float16)          # [8192, D]
    wt16 = np.ascontiguousarray(
        np.asarray(W_block, dtype=np.float32).T.astype(np.float16))
    in_maps = []
    for c in range(N_CORES):
        chunk = sel16[c * TPC:(c + 1) * TPC]         # [TPC, D]
        in_maps.append({
            "xs": np.ascontiguousarray(chunk.T),     # [D, TPC] feature-major
            "w": wt16,
        })
    res = run_bass_kernel_spmd(nc, in_maps, core_ids=list(range(N_CORES)),
                               trace=trace)
    out = x.copy()
    outf = out.reshape(B * S, D)
    for c in range(N_CORES):
        yo = res.results[c]["yo"]                    # [D, TPC] f16
        outf[sel_rows[c * TPC:(c + 1) * TPC]] = yo.T.astype(np.float32)
    return out, res


def kernel(x, W_block, W_router, top_k):
    assert int(top_k) == K_TOP, f"kernel compiled for top_k={K_TOP}, got {top_k}"
    trace = bool(os.environ.get("MOD_TRACE"))
    out, _ = run(x, W_block, W_router, trace=trace)
    return out


# revision 10
# speedup vs baseline: 1.2655x; 1.0328x over previous
"""Mixture-of-Depths routing kernel for Trainium2 (8 NeuronCores, SPMD).

Problem (per batch row b of 4):
    logits = x[b] @ W_router.T            # [4096]
    idx    = top_k(logits, 2048)          # half the tokens
    out[b] = x[b]; out[b][idx] = x[b][idx] @ W_block.T

Strategy: routing (router logits, top-k, gather, scatter) is pure data
movement / O(S*D) work and runs on the host in exact fp32 — the top-k
set it produces matches the reference's (the boundary gap between the
K-th and (K+1)-th logit is ~5e-4 for every row, orders of magnitude
above fp32 matmul noise ~1e-6). Passthrough tokens are copied from the
original fp32 x, so they are EXACT.

The device does the one irreducible O(K*D^2) piece: a dense Linear over
the 8192 selected tokens, sharded 1024 tokens per core (exactly
balanced since top-k is a fixed count per row). Each core runs
y^T = W x^T with features on psum partitions and tokens on the free
axis, fp16 inputs / fp32 accumulate (rel err ~4e-4 vs the 2e-2 gate).

Schedule (trace-tuned):
- phase 1 is k-outer over output chunks fs0-3 across all 8 psum banks,
  so the PE consumes each arriving xs chunk at ~the DMA cadence and
  never stalls on the input stream; phase 2 (fs4-7) runs fs-outer with
  everything resident. PE density measured ~100% (216ns/512-col matmul).
- first input tiles are split/ordered so the first matmul only needs
  128KB+128KB; phase-2 weight halves stream behind phase-1 halves.
- psum is drained to fp16 by scalar+vector alternately; output DMAs go
  out per half-chunk on the sync/gpsimd queues as soon as drained.
- the PE starts HAM-throttled at half clock (4/8 gate); ~20 dummy
  matmuls on a scratch tile during the input stream warm it up so the
  real matmuls run at full rate from the first one.
- the Tile epilogue's redundant second barrier round is stripped
  post-build (round 1 already waits on every DMA-completion semaphore).
PE time = 8192*1024*1024 MACs / 8 cores = 65536 cycles (~27.6us
@2.4GHz); DMA = 6MB/core fully hidden behind the PE.
"""
import os

import numpy as np

B, S, D = 4, 4096, 1024
K_TOP = 2048
N_CORES = 8
TPC = (B * K_TOP) // N_CORES   # 1024 selected tokens per core
NK = D // 128                  # 8 contraction / feature chunks
NG = TPC // 512                # 2 token groups of 512 (max moving free dim)
NF1 = 4                        # fs chunks in phase 1 (2 psum banks each)
N_WARM = 16                    # HAM warm-up dummy matmuls

_cache: dict = {}


def _build_nc():
    import concourse.bass as bass
    import concourse.mybir as mybir
    from concourse.tile import TileContext

    class _SplitWaitTC(TileContext):
        """The walrus build in this container rejects instructions carrying
        more than one sync-wait command. Tile's wait assignment routinely
        attaches several. After scheduling, move excess waits onto
        single-wait NoOps inserted before the instruction on the same
        engine (engine streams execute in order, so semantics are kept)."""

        def __exit__(self, exc_type, exc_value, traceback):
            r = super().__exit__(exc_type, exc_value, traceback)
            if exc_type is None:
                uid = 0
                for fn in self.nc.m.functions:
                    for bb in fn.blocks:
                        out = []
                        for inst in bb.instructions:
                            si = inst.sync_info
                            if si is not None and len(si.on_wait) > 1:
                                waits = list(si.on_wait)
                                si.on_wait = waits[-1:]
                                for w in waits[:-1]:
                                    uid += 1
                                    out.append(
                                        mybir.InstNoOp(
                                            name=f"I-waitsplit-{uid}",
                                            engine=inst.engine,
                                            ins=[],
                                            outs=[],
                                            sync_info=mybir.SyncInfo(
                                                on_wait=[w], on_update=[]
                                            ),
                                            text_hint="waitsplit",
                                            bass_nofuse=True,
                                        )
                                    )
                            out.append(inst)
                        bb.instructions = out
                # strip the epilogue's second barrier round: round 1 already
                # waits on every engine + DMA-queue completion semaphore
                for fn in self.nc.m.functions:
                    bb = fn.blocks[-1]
                    insts = bb.instructions
                    pool_rel = [
                        idx for idx, i in enumerate(insts)
                        if isinstance(i, mybir.InstEventSemaphore)
                        and i.engine == mybir.EngineType.Pool
                        and i.sync_info is not None
                        and any(u.ant_name.endswith("_release")
                                for u in i.sync_info.on_update)]
                    if len(pool_rel) == 2:
                        bb.instructions = (insts[:pool_rel[0] + 1]
                                           + insts[pool_rel[1] + 1:])
            return r

    f32 = mybir.dt.float32
    f16 = mybir.dt.float16

    nc = bass.Bass("TRN2", target_bir_lowering=False, debug=False,
                   num_devices=N_CORES)
    xs_d = nc.dram_tensor("xs", [D, TPC], f16, kind="ExternalInput")
    w_d = nc.dram_tensor("w", [D, D], f16, kind="ExternalInput")
    yo_d = nc.dram_tensor("yo", [D, TPC], f16, kind="ExternalOutput")

    with _SplitWaitTC(nc) as tc:
        with (
            tc.tile_pool(name="xpool", bufs=1) as xpool,
            tc.tile_pool(name="wpool", bufs=1) as wpool,
            tc.tile_pool(name="cpool", bufs=1) as cpool,
            tc.tile_pool(name="opool", bufs=8) as opool,
            tc.tile_pool(name="mm_pool", bufs=8, space="PSUM") as mm_pool,
        ):
            xs = [xpool.tile([128, TPC], f16, name=f"xs{k}") for k in range(NK)]
            wt = [wpool.tile([128, D], f16, name=f"wt{k}") for k in range(NK)]
            # sync ring: xs stream, first chunk split so the first matmul
            # only waits on 128KB
            nc.sync.dma_start(out=xs[0][:, 0:512], in_=xs_d[0:128, 0:512])
            nc.sync.dma_start(out=xs[0][:, 512:1024], in_=xs_d[0:128, 512:1024])
            for k in range(1, NK):
                nc.sync.dma_start(out=xs[k][:], in_=xs_d[k * 128:(k + 1) * 128, :])
            # gpsimd ring: phase-1 weight halves (cols 0:512 = fs0-3) first,
            # phase-2 halves stream behind
            for k in range(NK):
                nc.gpsimd.dma_start(out=wt[k][:, 0:512],
                                    in_=w_d[k * 128:(k + 1) * 128, 0:512])
            for k in range(NK):
                nc.gpsimd.dma_start(out=wt[k][:, 512:1024],
                                    in_=w_d[k * 128:(k + 1) * 128, 512:1024])
            # first scalar op triggers a ~1.3us ACT table load; eat it during
            # the input stream, off the critical path
            warm = cpool.tile([1, 128], f32)
            nc.vector.memset(warm[:], 0.0)
            warm2 = cpool.tile([1, 128], f32)
            nc.scalar.copy(out=warm2[:], in_=warm[:])
            dum = cpool.tile([128, 256], f16)
            nc.vector.memset(dum[:], 0.25)

            def drain_store(fs, ps):
                # psum -> fp16 sbuf on scalar/vector, halves out on the
                # sync/gpsimd queues as soon as each is drained
                of = opool.tile([128, TPC], f16, name="of")
                for g in range(NG):
                    dst = of[:, g * 512:(g + 1) * 512]
                    if (fs + g) % 2 == 0:
                        nc.scalar.copy(out=dst, in_=ps[g][:])
                    else:
                        nc.vector.tensor_copy(out=dst, in_=ps[g][:])
                for g in range(NG):
                    eng = nc.sync if (fs + g) % 2 == 0 else nc.gpsimd
                    eng.dma_start(
                        out=yo_d[fs * 128:(fs + 1) * 128,
                                 g * 512:(g + 1) * 512],
                        in_=of[:, g * 512:(g + 1) * 512])

            # phase 1: k-outer over fs0-3 across all 8 psum banks
            ps1 = [[mm_pool.tile([128, 512], f32, name="ps")
                    for _ in range(NG)] for _ in range(NF1)]
            # HAM warm-up: dummy matmuls while the inputs stream (results
            # discarded; the first real matmul resets the bank via start)
            for _ in range(N_WARM):
                nc.tensor.matmul(out=ps1[0][0][:, 0:256],
                                 lhsT=dum[:, 0:128], rhs=dum[:, :],
                                 start=True, stop=True)
            for k in range(NK):
                for fs in range(NF1):
                    for g in range(NG):
                        nc.tensor.matmul(
                            out=ps1[fs][g][:],
                            lhsT=wt[k][:, fs * 128:(fs + 1) * 128],
                            rhs=xs[k][:, g * 512:(g + 1) * 512],
                            start=(k == 0), stop=(k == NK - 1))
            for fs in range(NF1):
                drain_store(fs, ps1[fs])

            # phase 2: fs-outer for fs4-7; psum banks recycle as the
            # phase-1 drains complete
            for fs in range(NF1, NK):
                ps = [mm_pool.tile([128, 512], f32, name="ps")
                      for _ in range(NG)]
                for k in range(NK):
                    for g in range(NG):
                        nc.tensor.matmul(
                            out=ps[g][:],
                            lhsT=wt[k][:, fs * 128:(fs + 1) * 128],
                            rhs=xs[k][:, g * 512:(g + 1) * 512],
                            start=(k == 0), stop=(k == NK - 1))
                drain_store(fs, ps)
    return nc


def _get_nc():
    if "nc" not in _cache:
        _cache["nc"] = _build_nc()
    return _cache["nc"]


def _route(x, W_router):
    """Host-side routing: exact fp32 logits -> per-row top-k index set."""
    wr = np.asarray(W_router, dtype=np.float32).reshape(D)
    logits = (x.reshape(B * S, D) @ wr).reshape(B, S)
    rows = []
    for b in range(B):
        idx = np.argpartition(logits[b], S - K_TOP)[S - K_TOP:]
        idx.sort()
        rows.append(b * S + idx)
    return np.concatenate(rows)          # [B*K_TOP] flat selected rows


def run(x, W_block, W_router, trace=False):
    from concourse.bass_utils import run_bass_kernel_spmd

    nc = _get_nc()
    x = np.asarray(x, dtype=np.float32)
    sel_rows = _route(x, W_router)
    xf = x.reshape(B * S, D)
    sel16 = xf[sel_rows].astype(np.float16)          # [8192, D]
    wt16 = np.ascontiguousarray(
        np.asarray(W_block, dtype=np.float32).T.astype(np.float16))
    in_maps = []
    for c in range(N_CORES):
        chunk = sel16[c * TPC:(c + 1) * TPC]         # [TPC, D]
        in_maps.append({
            "xs": np.ascontiguousarray(chunk.T),     # [D, TPC] feature-major
            "w": wt16,
        })
    res = run_bass_kernel_spmd(nc, in_maps, core_ids=list(range(N_CORES)),
                               trace=trace)
    out = x.copy()
    outf = out.reshape(B * S, D)
    for c in range(N_CORES):
        yo = res.results[c]["yo"]                    # [D, TPC] f16
        outf[sel_rows[c * TPC:(c + 1) * TPC]] = yo.T.astype(np.float32)
    return out, res


def kernel(x, W_block, W_router, top_k):
    assert int(top_k) == K_TOP, f"kernel compiled for top_k={K_TOP}, got {top_k}"
    trace = bool(os.environ.get("MOD_TRACE"))
    out, _ = run(x, W_block, W_router, trace=trace)
    return out
